# revision 8
# baseline (speedup 1.0000x reference)
"""Deep Richardson-Lucy deconvolution on 8 Trainium2 NeuronCores.

Strategy (per core, data-parallel batch shard of 512 rows):
- Everything lives in SBUF in a TRANSPOSED fp16 layout: [l on partitions
  (64 blocks of 128), batch on the free dim (512)].
- conv1d(K=31, zero-pad) == banded-Toeplitz matmul per 128-l block:
  one center [128,128] matmul + two 32-row halo matmuls against the
  neighbor blocks, packed to array corners via tile_position.
- Per RL iteration: conv(s) -> PSUM; r = ACT.Reciprocal(psum + EPS);
  ratio = m * r (DVE fp16 2x); conv(ratio, flipped) -> PSUM;
  s *= psum (DVE, PSUM operand).
- In/out transposes ride the DMA xbar transpose engine (fp16).
"""
import hashlib
import numpy as np

EPS = 1e-6
P = 128
KTAPS = 31
PAD = 15
B_FULL, L = 4096, 8192
N_CORES = 8
BC = B_FULL // N_CORES          # 512 batch rows per core
NT = L // P                     # 64 l-blocks
NITER = 10

_cache = {}


def _build_toeplitz(psf):
    Wc = np.zeros((P, P), dtype=np.float64)
    j = np.arange(P)[:, None]
    i = np.arange(P)[None, :]
    k = j - i + PAD
    m = (k >= 0) & (k < KTAPS)
    Wc[m] = psf[k[m]]
    WL = np.zeros((32, 32), dtype=np.float64)   # rhs = prev block parts [96,128)
    jj = np.arange(32)[:, None]
    ii = np.arange(32)[None, :]
    k = (96 + jj - 128) - ii + PAD
    m = (k >= 0) & (k < KTAPS)
    WL[m] = psf[k[m]]
    WR = np.zeros((32, 32), dtype=np.float64)   # rhs = next block parts [0,32)
    k = (jj + 128) - (96 + ii) + PAD
    m = (k >= 0) & (k < KTAPS)
    WR[m] = psf[k[m]]
    return Wc, WL, WR


def _wpack(psf):
    """[P, 256] fp16: cols 0:128 = center Toeplitz; cols 128:256 = combined
    halo weight (rows 0:32 = WR mapping next-block rows 0:32 -> out 96:128;
    rows 32:64 = WL mapping prev-block rows 96:128 -> out 0:32)."""
    Wc, WL, WR = _build_toeplitz(psf)
    w = np.zeros((P, 256), dtype=np.float16)
    w[:, 0:128] = Wc
    w[0:32, 128 + 96:128 + 128] = WR
    w[32:64, 128 + 0:128 + 32] = WL
    return w


def _r0pack(psf64):
    """r0[p, t] = 1 / (conv1d(0.5*ones, psf)[128t+p] + EPS)."""
    ones = np.full((1, L), 0.5, dtype=np.float64)
    xp = np.pad(ones, ((0, 0), (PAD, PAD)))
    sc = np.zeros((1, L), dtype=np.float64)
    for k in range(KTAPS):
        sc += xp[:, k:k + L] * psf64[k]
    r = 1.0 / (sc[0] + EPS)
    return r.reshape(NT, P).T.astype(np.float32)


def _build(psf64, alpha64):
    import concourse.bass as bass
    import concourse.tile as tile
    from concourse import mybir
    import bass_rust

    F32 = mybir.dt.float32
    F16 = mybir.dt.float16

    class SafeTC(tile.TileContext):
        # this walrus build rejects >1 sync wait per CTRL-class instruction
        def _drain_and_barrier(self, tick_clock, wait_clock):
            gc = tick_clock.global_clock
            for i in range(len(gc)):
                if gc[i] > 0:
                    di = self.nc.sync.drain()
                    pc = bass_rust.VectorClock()
                    pc.require_at_least(i, gc[i])
                    wait_clock.add_sem_waits(di.ins, bass_rust.ScopedClock({None: pc}))
            self.nc.all_engine_barrier()
            popped = self.nc._tile_sem_poison_stack.pop()
            assert popped is self._sem_poison
            self.nc.clear_and_free_semaphores(list(self.sems.allocated().values()))
            self.nc.all_engine_barrier()

    def split_multi_waits(nc, max_waits=1):
        n_fixed = 0
        uid = [0]
        for f in nc.m.functions:
            for bb in f.blocks:
                out = []
                changed = False
                for inst in bb.instructions:
                    si = inst.sync_info
                    if si is not None:
                        sems = [w for w in si.on_wait
                                if str(getattr(w, "sync_type", "")) == "semaphore"]
                        other = [w for w in si.on_wait if w not in sems]
                        if len(sems) > max_waits:
                            keep = sems[-max_waits:]
                            for w in sems[:-max_waits]:
                                nop = mybir.InstNoOp(
                                    name=f"waitsplit_{uid[0]}", ins=[], outs=[])
                                uid[0] += 1
                                nop.engine = inst.engine
                                nop.sync_info = mybir.SyncInfo(
                                    on_wait=[w], on_update=[])
                                out.append(nop)
                            inst.sync_info = mybir.SyncInfo(
                                on_wait=other + keep,
                                on_update=list(si.on_update))
                            n_fixed += 1
                            changed = True
                    out.append(inst)
                if changed:
                    try:
                        bb.instructions = out
                    except Exception:
                        bb.instructions.clear()
                        bb.instructions.extend(out)
        return n_fixed

    def act_raw(nc, out, in_, func, bias=0.0, scale=1.0):
        eng = nc.scalar
        ins = [eng.lower_ap(in_),
               mybir.ImmediateValue(dtype=F32, value=float(bias)),
               mybir.ImmediateValue(dtype=F32, value=float(scale)),
               mybir.ImmediateValue(dtype=F32, value=0.0)]
        return eng.add_instruction(mybir.InstActivation(
            name=nc.get_next_instruction_name(), func=func, ins=ins,
            outs=[eng.lower_ap(out)]))

    alpha_is_one = bool(np.all(alpha64 == 1.0))

    nc = bass.Bass("TRN2", target_bir_lowering=False, debug=False,
                   num_devices=N_CORES)
    m_in = nc.dram_tensor("m", [BC, L], F32, kind="ExternalInput")
    w1_in = nc.dram_tensor("w1", [P, 256], F16, kind="ExternalInput")
    w2_in = nc.dram_tensor("w2", [P, 256], F16, kind="ExternalInput")
    r0_in = nc.dram_tensor("r0", [P, NT], F32, kind="ExternalInput")
    y_out = nc.dram_tensor("y", [BC, L], F32, kind="ExternalOutput")

    Rec = mybir.ActivationFunctionType.Reciprocal
    Ln = mybir.ActivationFunctionType.Ln
    Exp = mybir.ActivationFunctionType.Exp

    def conv_block(psum, w, center_src, t, halo):
        """Center matmul + ONE combined-halo matmul against the packed
        [64, BC] halo tile (rows 0:32 = next-block rows 0:32 via WR,
        rows 32:64 = prev-block rows 96:128 via WL)."""
        nc.tensor.matmul(psum[:], w[:, 0:128], center_src,
                         start=True, stop=False)
        if t == 0:
            nc.tensor.matmul(psum[:], w[0:32, 128:256], halo[0:32, :],
                             start=False, stop=True)
        elif t == NT - 1:
            nc.tensor.matmul(psum[:], w[32:64, 128:256], halo[32:64, :],
                             start=False, stop=True, tile_position=(32, 0))
        else:
            nc.tensor.matmul(psum[:], w[0:64, 128:256], halo[0:64, :],
                             start=False, stop=True)

    def halo_copies(dst, src, t):
        """DMA-pack neighbor rows for block t into dst [64, BC]."""
        if t < NT - 1:
            nc.sync.dma_start(dst[0:32, :], src[0:32, t + 1, :])
        if t > 0:
            nc.sync.dma_start(dst[32:64, :], src[96:128, t - 1, :])

    with SafeTC(nc) as tc:
        with tc.tile_pool(name="wpool", bufs=1) as wpool, \
             tc.tile_pool(name="mpool", bufs=1) as mpool, \
             tc.tile_pool(name="spool", bufs=1) as spool:
            w1 = wpool.tile([P, 256], F16)
            nc.sync.dma_start(w1[:], w1_in[:])
            w2 = wpool.tile([P, 256], F16)
            nc.sync.dma_start(w2[:], w2_in[:])
            r0 = wpool.tile([P, NT], F32)
            nc.sync.dma_start(r0[:], r0_in[:])
            mT = mpool.tile([P, NT, BC], F16)
            s = spool.tile([P, NT, BC], F16)
            nc.vector.memset(s[:], 0.5)

            # ---- load m, cast fp16, DMA-xbar transpose into mT ----
            with tc.tile_pool(name="stage", bufs=1) as stage:
                for c in range(BC // P):
                    st32 = stage.tile([P, L], F32, tag="st32")
                    nc.sync.dma_start(st32[:], m_in[c * P:(c + 1) * P, :])
                    st16 = stage.tile([P, L], F16, tag="st16")
                    nc.vector.tensor_copy(st16[:], st32[:])
                    nc.sync.dma_start_transpose(
                        mT[:, :, c * P:(c + 1) * P], st16[:])

            # ---- RL iterations ----
            with tc.tile_pool(name="ratio", bufs=8) as rpool, \
                 tc.tile_pool(name="rtile", bufs=4) as rtp, \
                 tc.tile_pool(name="h1p", bufs=6) as h1p, \
                 tc.tile_pool(name="h2p", bufs=6) as h2p, \
                 tc.tile_pool(name="psum", bufs=8, space="PSUM") as pp:
                for it in range(NITER):
                    ratio_tiles = [None] * NT
                    h1_tiles = [None] * NT
                    h2_tiles = [None] * NT

                    def _h1(t):
                        # conv1 halo pack for block t of THIS iter: reads s
                        # written by last iter's updates of t-1 / t+1 (both
                        # long since done when emitted >=2 blocks ahead).
                        ht = h1p.tile([64, BC], F16, tag="h1")
                        halo_copies(ht, s, t)
                        h1_tiles[t] = ht

                    def _h2(t):
                        # conv2 halo pack: reads ratio tiles t-1 / t+1.
                        ht = h2p.tile([64, BC], F16, tag="h2")
                        if t < NT - 1:
                            nc.sync.dma_start(ht[0:32, :],
                                              ratio_tiles[t + 1][0:32, :])
                        if t > 0:
                            nc.sync.dma_start(ht[32:64, :],
                                              ratio_tiles[t - 1][96:128, :])
                        h2_tiles[t] = ht

                    def _ratio(t):
                        ra = rpool.tile([P, BC], F16, tag="ra")
                        if it == 0:
                            # s == 0.5 everywhere: conv(s)+EPS is a per-l
                            # constant; r0 = 1/that, precomputed on host.
                            nc.vector.tensor_scalar(
                                out=ra[:], in0=mT[:, t, :],
                                scalar1=r0[:, t:t + 1], scalar2=None,
                                op0=mybir.AluOpType.mult)
                        else:
                            ps = pp.tile([P, BC], mybir.dt.float32, tag="ps")
                            conv_block(ps, w1, s[:, t, :], t, h1_tiles[t])
                            rt = rtp.tile([P, BC], F16, tag="rt")
                            act_raw(nc, rt[:], ps[:], Rec, bias=EPS)
                            nc.vector.tensor_mul(ra[:], mT[:, t, :], rt[:])
                        ratio_tiles[t] = ra

                    def _conv2_update(t):
                        ps = pp.tile([P, BC], mybir.dt.float32, tag="ps")
                        conv_block(ps, w2, ratio_tiles[t][:], t, h2_tiles[t])
                        if alpha_is_one:
                            if t % 2 == 0:
                                # DVE fused: s = (psum + EPS) * s, PSUM src 1x
                                nc.vector.scalar_tensor_tensor(
                                    out=s[:, t, :], in0=ps[:], scalar=EPS,
                                    in1=s[:, t, :],
                                    op0=mybir.AluOpType.add,
                                    op1=mybir.AluOpType.mult)
                            else:
                                # ACT evacuates PSUM (+EPS), DVE fp16 mul 2x
                                cp = rtp.tile([P, BC], F16, tag="cp")
                                act_raw(nc, cp[:], ps[:],
                                        mybir.ActivationFunctionType.Copy,
                                        bias=EPS)
                                nc.vector.tensor_mul(s[:, t, :], s[:, t, :],
                                                     cp[:])
                        else:
                            lg = rtp.tile([P, BC], F32, tag="lg")
                            act_raw(nc, lg[:], ps[:], Ln, bias=EPS)
                            cp = rtp.tile([P, BC], F16, tag="cp")
                            act_raw(nc, cp[:], lg[:], Exp,
                                    scale=float(alpha64[it]))
                            nc.vector.tensor_mul(s[:, t, :], s[:, t, :], cp[:])

    # software-pipelined emission. Leads: h1 DMA 7 blocks ahead of
                    # its conv1, ratio 4 ahead of conv2, h2 DMA 3 ahead of
                    # its conv2 — so no matmul ever waits on a DMA or an
                    # elementwise producer.
                    if it > 0:
                        for t in range(7):
                            _h1(t)
                    for t in range(4):
                        _ratio(t)
                    for t in range(3):
                        _h2(t)
                    for w in range(NT):
                        if it > 0 and w + 7 < NT:
                            _h1(w + 7)
                        if w + 4 < NT:
                            _ratio(w + 4)
                        if w + 3 < NT:
                            _h2(w + 3)
                        _conv2_update(w)

            # ---- transpose back + cast fp32 + store ----
            with tc.tile_pool(name="outp", bufs=1) as outp:
                for q in range(4):
                    sn16 = outp.tile([P, NT, P], F16, tag="sn16")
                    nc.sync.dma_start_transpose(sn16[:], s[:, q * 16:(q + 1) * 16, :])
                    sn32 = outp.tile([P, NT, P], F32, tag="sn32")
                    nc.vector.tensor_copy(sn32[:], sn16[:])
                    sn32r = sn32.rearrange("p (tl bc) lp -> p tl bc lp",
                                           tl=16, bc=4)
                    for bc in range(4):
                        ydst = y_out[bc * P:(bc + 1) * P,
                                     q * 2048:(q + 1) * 2048].rearrange(
                            "p (tl lp) -> p tl lp", lp=P)
                        nc.sync.dma_start(ydst, sn32r[:, :, bc, :])

    split_multi_waits(nc)
    return nc


def _make_in_maps(m, psf, alpha):
    m = np.asarray(m)
    psf64 = np.asarray(psf, dtype=np.float64)
    w1 = _wpack(psf64)
    w2 = _wpack(psf64[::-1])
    r0 = _r0pack(psf64)
    return [{"m": np.ascontiguousarray(m[c * BC:(c + 1) * BC]).astype(np.float32),
             "w1": w1, "w2": w2, "r0": r0} for c in range(N_CORES)]


def kernel(m, psf, alpha):
    m = np.asarray(m)
    psf64 = np.asarray(psf, dtype=np.float64)
    alpha64 = np.asarray(alpha, dtype=np.float64)
    key = hashlib.sha256(
        psf64.tobytes() + alpha64.tobytes() + str(m.shape).encode()).hexdigest()
    if key not in _cache:
        _cache[key] = _build(psf64, alpha64)
    nc = _cache[key]

    from concourse.bass_utils import run_bass_kernel_spmd
    in_maps = _make_in_maps(m, psf, alpha)
    res = run_bass_kernel_spmd(nc, in_maps, core_ids=list(range(N_CORES)))
    out = np.concatenate([res.results[c]["y"] for c in range(N_CORES)], axis=0)
    return out.astype(np.float32)



# revision 12
# speedup vs baseline: 1.9746x; 1.9746x over previous
"""Deep Richardson-Lucy deconvolution on 8 Trainium2 NeuronCores.

Strategy (per core, data-parallel batch shard of 512 rows):
- Everything lives in SBUF in a TRANSPOSED fp16 layout: [l on partitions
  (64 blocks of 128), batch on the free dim (512)].
- conv1d(K=31, zero-pad) == banded-Toeplitz matmul per 128-l block:
  one center [128,128] matmul + two 32-row halo matmuls against the
  neighbor blocks, packed to array corners via tile_position.
- Per RL iteration: conv(s) -> PSUM; r = ACT.Reciprocal(psum + EPS);
  ratio = m * r (DVE fp16 2x); conv(ratio, flipped) -> PSUM;
  s *= psum (DVE, PSUM operand).
- In/out transposes ride the DMA xbar transpose engine (fp16).
"""
import hashlib
import numpy as np

EPS = 1e-6
P = 128
KTAPS = 31
PAD = 15
B_FULL, L = 4096, 8192
N_CORES = 8
BC = B_FULL // N_CORES          # 512 batch rows per core
NT = L // P                     # 64 l-blocks
NITER = 10

_cache = {}


def _build_toeplitz(psf):
    Wc = np.zeros((P, P), dtype=np.float64)
    j = np.arange(P)[:, None]
    i = np.arange(P)[None, :]
    k = j - i + PAD
    m = (k >= 0) & (k < KTAPS)
    Wc[m] = psf[k[m]]
    WL = np.zeros((32, 32), dtype=np.float64)   # rhs = prev block parts [96,128)
    jj = np.arange(32)[:, None]
    ii = np.arange(32)[None, :]
    k = (96 + jj - 128) - ii + PAD
    m = (k >= 0) & (k < KTAPS)
    WL[m] = psf[k[m]]
    WR = np.zeros((32, 32), dtype=np.float64)   # rhs = next block parts [0,32)
    k = (jj + 128) - (96 + ii) + PAD
    m = (k >= 0) & (k < KTAPS)
    WR[m] = psf[k[m]]
    return Wc, WL, WR


def _wpack(psf):
    """[P, 256] fp16: cols 0:128 = center Toeplitz; cols 128:256 = halo
    weight usable BOTH as one combined [128,128] matmul (against an H-pack
    tile whose rows 0:32 = next-block rows 0:32, rows 96:128 = prev-block
    rows 96:128, middle zero) AND as two separate 32-row matmuls
    (w[0:32,224:256]=WR -> out 96:128; w[96:128,128:160]=WL -> out 0:32)."""
    Wc, WL, WR = _build_toeplitz(psf)
    w = np.zeros((P, 256), dtype=np.float16)
    w[:, 0:128] = Wc
    w[0:32, 128 + 96:128 + 128] = WR
    w[96:128, 128 + 0:128 + 32] = WL
    return w


def _r0pack(psf64):
    """r0[p, t] = 1 / (conv1d(0.5*ones, psf)[128t+p] + EPS)."""
    ones = np.full((1, L), 0.5, dtype=np.float64)
    xp = np.pad(ones, ((0, 0), (PAD, PAD)))
    sc = np.zeros((1, L), dtype=np.float64)
    for k in range(KTAPS):
        sc += xp[:, k:k + L] * psf64[k]
    r = 1.0 / (sc[0] + EPS)
    return r.reshape(NT, P).T.astype(np.float32)


def _build(psf64, alpha64):
    import concourse.bass as bass
    import concourse.tile as tile
    from concourse import mybir
    import bass_rust

    F32 = mybir.dt.float32
    F16 = mybir.dt.float16

    class SafeTC(tile.TileContext):
        # this walrus build rejects >1 sync wait per CTRL-class instruction
        def _drain_and_barrier(self, tick_clock, wait_clock):
            gc = tick_clock.global_clock
            for i in range(len(gc)):
                if gc[i] > 0:
                    di = self.nc.sync.drain()
                    pc = bass_rust.VectorClock()
                    pc.require_at_least(i, gc[i])
                    wait_clock.add_sem_waits(di.ins, bass_rust.ScopedClock({None: pc}))
            self.nc.all_engine_barrier()
            popped = self.nc._tile_sem_poison_stack.pop()
            assert popped is self._sem_poison
            self.nc.clear_and_free_semaphores(list(self.sems.allocated().values()))
            self.nc.all_engine_barrier()

    def split_multi_waits(nc, max_waits=1):
        n_fixed = 0
        uid = [0]
        for f in nc.m.functions:
            for bb in f.blocks:
                out = []
                changed = False
                for inst in bb.instructions:
                    si = inst.sync_info
                    if si is not None:
                        sems = [w for w in si.on_wait
                                if str(getattr(w, "sync_type", "")) == "semaphore"]
                        other = [w for w in si.on_wait if w not in sems]
                        if len(sems) > max_waits:
                            keep = sems[-max_waits:]
                            for w in sems[:-max_waits]:
                                nop = mybir.InstNoOp(
                                    name=f"waitsplit_{uid[0]}", ins=[], outs=[])
                                uid[0] += 1
                                nop.engine = inst.engine
                                nop.sync_info = mybir.SyncInfo(
                                    on_wait=[w], on_update=[])
                                out.append(nop)
                            inst.sync_info = mybir.SyncInfo(
                                on_wait=other + keep,
                                on_update=list(si.on_update))
                            n_fixed += 1
                            changed = True
                    out.append(inst)
                if changed:
                    try:
                        bb.instructions = out
                    except Exception:
                        bb.instructions.clear()
                        bb.instructions.extend(out)
        return n_fixed

    def act_raw(nc, out, in_, func, bias=0.0, scale=1.0):
        eng = nc.scalar
        ins = [eng.lower_ap(in_),
               mybir.ImmediateValue(dtype=F32, value=float(bias)),
               mybir.ImmediateValue(dtype=F32, value=float(scale)),
               mybir.ImmediateValue(dtype=F32, value=0.0)]
        return eng.add_instruction(mybir.InstActivation(
            name=nc.get_next_instruction_name(), func=func, ins=ins,
            outs=[eng.lower_ap(out)]))

    alpha_is_one = bool(np.all(alpha64 == 1.0))

    nc = bass.Bass("TRN2", target_bir_lowering=False, debug=False,
                   num_devices=N_CORES)
    m_in = nc.dram_tensor("m", [BC, L], F32, kind="ExternalInput")
    w1_in = nc.dram_tensor("w1", [P, 256], F16, kind="ExternalInput")
    w2_in = nc.dram_tensor("w2", [P, 256], F16, kind="ExternalInput")
    r0_in = nc.dram_tensor("r0", [P, NT], F32, kind="ExternalInput")
    y_out = nc.dram_tensor("y", [BC, L], F32, kind="ExternalOutput")

    Rec = mybir.ActivationFunctionType.Reciprocal
    Ln = mybir.ActivationFunctionType.Ln
    Exp = mybir.ActivationFunctionType.Exp

    RING = 16                 # H1 ring columns
    CH = 4                    # blocks per H1 bulk-copy chunk

    def conv1_block(psum, w, s, h1ring, t):
        """Center matmul + ONE combined-halo matmul against the H1 ring
        column (rows 0:32 = s rows 0:32 of block t+1, rows 96:128 =
        s rows 96:128 of block t-1, middle rows permanently zero)."""
        r = t % RING
        nc.tensor.matmul(psum[:], w[:, 0:128], s[:, t, :],
                         start=True, stop=False)
        if t == 0:
            nc.tensor.matmul(psum[:], w[0:32, 128:256], h1ring[0:32, r, :],
                             start=False, stop=True)
        elif t == NT - 1:
            nc.tensor.matmul(psum[:], w[96:128, 128:256],
                             h1ring[96:128, r, :],
                             start=False, stop=True, tile_position=(96, 0))
        else:
            nc.tensor.matmul(psum[:], w[:, 128:256], h1ring[:, r, :],
                             start=False, stop=True)

    def conv2_block(psum, w, ratio_tiles, t):
        """V1-style: center + two 32-row halo matmuls reading the ratio
        pool tiles' partition subranges directly (no copies)."""
        last = "R" if t < NT - 1 else "L"
        nc.tensor.matmul(psum[:], w[:, 0:128], ratio_tiles[t][:],
                         start=True, stop=False)
        if t > 0:
            nc.tensor.matmul(psum[0:32, :], w[96:128, 128:160],
                             ratio_tiles[t - 1][96:128, :], start=False,
                             stop=(last == "L"), tile_position=(96, 0))
        if t < NT - 1:
            nc.tensor.matmul(psum[96:128, :], w[0:32, 224:256],
                             ratio_tiles[t + 1][0:32, :], start=False,
                             stop=(last == "R"), tile_position=(0, 96))

    def h1_chunk(h1ring, s, c):
        """Bulk-copy H1 ring slots for blocks [4c, 4c+4): same-partition
        SBUF->SBUF DMAs (one per side), contiguous 4KB per partition."""
        t0, t1 = CH * c, min(CH * (c + 1), NT)
        r0 = (CH * c) % RING
        # top rows: s[0:32, t+1] for t in [t0, t1); block NT-1 never reads
        # its top slot (edge matmul), so clip the source at NT.
        lo, hi = t0 + 1, min(t1 + 1, NT)
        if hi > lo:
            nc.sync.dma_start(h1ring[0:32, r0:r0 + (hi - lo), :],
                              s[0:32, lo:hi, :])
        # bottom rows: s[96:128, t-1]; block 0 never reads its bottom slot.
        lo, hi = max(t0 - 1, 0), t1 - 1
        if hi > lo:
            rb = r0 if t0 > 0 else r0 + 1
            nc.sync.dma_start(h1ring[96:128, rb:rb + (hi - lo), :],
                              s[96:128, lo:hi, :])

    with SafeTC(nc) as tc:
        with tc.tile_pool(name="wpool", bufs=1) as wpool, \
             tc.tile_pool(name="mpool", bufs=1) as mpool, \
             tc.tile_pool(name="spool", bufs=1) as spool:
            w1 = wpool.tile([P, 256], F16)
            nc.sync.dma_start(w1[:], w1_in[:])
            w2 = wpool.tile([P, 256], F16)
            nc.sync.dma_start(w2[:], w2_in[:])
            r0 = wpool.tile([P, NT], F32)
            nc.sync.dma_start(r0[:], r0_in[:])
            mT = mpool.tile([P, NT, BC], F16)
            s = spool.tile([P, NT, BC], F16)
            nc.vector.memset(s[:], 0.5)

            # ---- load m, cast fp16, DMA-xbar transpose into mT ----
            with tc.tile_pool(name="stage", bufs=1) as stage:
                for c in range(BC // P):
                    st32 = stage.tile([P, L], F32, tag="st32")
                    nc.sync.dma_start(st32[:], m_in[c * P:(c + 1) * P, :])
                    st16 = stage.tile([P, L], F16, tag="st16")
                    nc.vector.tensor_copy(st16[:], st32[:])
                    nc.sync.dma_start_transpose(
                        mT[:, :, c * P:(c + 1) * P], st16[:])

            # ---- RL iterations ----
            h1r = spool.tile([P, RING, BC], F16)
            nc.vector.memset(h1r[:], 0.0)
            with tc.tile_pool(name="ratio", bufs=8) as rpool, \
                 tc.tile_pool(name="rtile", bufs=4) as rtp, \
                 tc.tile_pool(name="psum", bufs=8, space="PSUM") as pp:
                for it in range(NITER):
                    ratio_tiles = [None] * NT

                    def _ratio(t):
                        ra = rpool.tile([P, BC], F16, tag="ra")
                        if it == 0:
                            # s == 0.5 everywhere: conv(s)+EPS is a per-l
                            # constant; r0 = 1/that, precomputed on host.
                            nc.vector.tensor_scalar(
                                out=ra[:], in0=mT[:, t, :],
                                scalar1=r0[:, t:t + 1], scalar2=None,
                                op0=mybir.AluOpType.mult)
                        else:
                            ps = pp.tile([P, BC], mybir.dt.float32, tag="ps")
                            conv1_block(ps, w1, s, h1r, t)
                            rt = rtp.tile([P, BC], F16, tag="rt")
                            act_raw(nc, rt[:], ps[:], Rec, bias=EPS)
                            nc.vector.tensor_mul(ra[:], mT[:, t, :], rt[:])
                        ratio_tiles[t] = ra

                    def _conv2_update(t):
                        ps = pp.tile([P, BC], mybir.dt.float32, tag="ps")
                        conv2_block(ps, w2, ratio_tiles, t)
                        if alpha_is_one:
                            if t % 2 == 0:
                                # DVE fused: s = (psum + EPS) * s, PSUM src 1x
                                nc.vector.scalar_tensor_tensor(
                                    out=s[:, t, :], in0=ps[:], scalar=EPS,
                                    in1=s[:, t, :],
                                    op0=mybir.AluOpType.add,
                                    op1=mybir.AluOpType.mult)
                            else:
                                # ACT evacuates PSUM (+EPS), DVE fp16 mul 2x
                                cp = rtp.tile([P, BC], F16, tag="cp")
                                act_raw(nc, cp[:], ps[:],
                                        mybir.ActivationFunctionType.Copy,
                                        bias=EPS)
                                nc.vector.tensor_mul(s[:, t, :], s[:, t, :],
                                                     cp[:])
                        else:
                            lg = rtp.tile([P, BC], F32, tag="lg")
                            act_raw(nc, lg[:], ps[:], Ln, bias=EPS)
                            cp = rtp.tile([P, BC], F16, tag="cp")
                            act_raw(nc, cp[:], lg[:], Exp,
                                    scale=float(alpha64[it]))
                            nc.vector.tensor_mul(s[:, t, :], s[:, t, :], cp[:])

    # software-pipelined emission. Leads: H1 ring chunks ~8
                    # blocks ahead of their conv1, ratio 4 ahead of conv2 —
                    # so no matmul waits on a DMA or elementwise producer.
                    if it > 0:
                        h1_chunk(h1r, s, 0)
                        h1_chunk(h1r, s, 1)
                    for t in range(4):
                        _ratio(t)
                    for w in range(NT):
                        if it > 0 and (w + 8) % CH == 0 and w + 8 < NT:
                            h1_chunk(h1r, s, (w + 8) // CH)
                        if w + 4 < NT:
                            _ratio(w + 4)
                        _conv2_update(w)

            # ---- transpose back + cast fp32 + store ----
            with tc.tile_pool(name="outp", bufs=1) as outp:
                for q in range(4):
                    sn16 = outp.tile([P, NT, P], F16, tag="sn16")
                    nc.sync.dma_start_transpose(sn16[:], s[:, q * 16:(q + 1) * 16, :])
                    sn32 = outp.tile([P, NT, P], F32, tag="sn32")
                    nc.vector.tensor_copy(sn32[:], sn16[:])
                    sn32r = sn32.rearrange("p (tl bc) lp -> p tl bc lp",
                                           tl=16, bc=4)
                    for bc in range(4):
                        ydst = y_out[bc * P:(bc + 1) * P,
                                     q * 2048:(q + 1) * 2048].rearrange(
                            "p (tl lp) -> p tl lp", lp=P)
                        nc.sync.dma_start(ydst, sn32r[:, :, bc, :])

    split_multi_waits(nc)
    return nc


def _make_in_maps(m, psf, alpha):
    m = np.asarray(m)
    psf64 = np.asarray(psf, dtype=np.float64)
    w1 = _wpack(psf64)
    w2 = _wpack(psf64[::-1])
    r0 = _r0pack(psf64)
    return [{"m": np.ascontiguousarray(m[c * BC:(c + 1) * BC]).astype(np.float32),
             "w1": w1, "w2": w2, "r0": r0} for c in range(N_CORES)]


def kernel(m, psf, alpha):
    m = np.asarray(m)
    psf64 = np.asarray(psf, dtype=np.float64)
    alpha64 = np.asarray(alpha, dtype=np.float64)
    key = hashlib.sha256(
        psf64.tobytes() + alpha64.tobytes() + str(m.shape).encode()).hexdigest()
    if key not in _cache:
        _cache[key] = _build(psf64, alpha64)
    nc = _cache[key]

    from concourse.bass_utils import run_bass_kernel_spmd
    in_maps = _make_in_maps(m, psf, alpha)
    res = run_bass_kernel_spmd(nc, in_maps, core_ids=list(range(N_CORES)))
    out = np.concatenate([res.results[c]["y"] for c in range(N_CORES)], axis=0)
    return out.astype(np.float32)



# revision 14
# speedup vs baseline: 2.3537x; 1.1920x over previous
"""Deep Richardson-Lucy deconvolution on 8 Trainium2 NeuronCores.

Strategy (per core, data-parallel batch shard of 512 rows):
- Everything lives in SBUF in a TRANSPOSED fp16 layout: [l on partitions
  (64 blocks of 128), batch on the free dim (512)]. The transpose/cast of
  m happens on the HOST (same fp16 rounding as on-chip), and the output
  un-transpose/fp32-cast also happens on the host, so the device does
  zero staging work: 4 chunked input DMAs, 4 chunked output DMAs.
- conv1d(K=31, zero-pad) == banded-Toeplitz matmul per 128-l block.
  conv1 (input s): center [128,128] matmul + ONE combined-halo matmul
  against an H1 ring tile filled by bulk same-partition SBUF->SBUF DMAs
  (rows 0:32 = next block's rows 0:32, rows 96:128 = prev block's rows
  96:128, middle rows permanently zero x zero weights).
  conv2 (input ratio): center + two 32-row halo matmuls reading the
  ratio tiles' partition subranges directly; the two halos land in
  disjoint PE quadrants (tile_position) and stream concurrently.
- Elementwise ops run on PAIRS of blocks (2-bank PSUM tiles, free size
  1024) to amortize per-instruction overhead and halve semaphore waits:
  r = ACT.Reciprocal(psum_pair + EPS); ratio = m * r (DVE fp16 2x);
  s *= (psum_pair + EPS) alternating DVE-stt / ACT-copy+DVE-mult.
- Deep software pipelining (ratio 2 pairs ahead, H1 chunks ~6 blocks
  ahead) keeps the PE 100% busy at full stream rate.
"""
import hashlib
import numpy as np

EPS = 1e-6
P = 128
KTAPS = 31
PAD = 15
B_FULL, L = 4096, 8192
N_CORES = 8
BC = B_FULL // N_CORES          # 512 batch rows per core
NT = L // P                     # 64 l-blocks
NP = NT // 2                    # 32 block pairs
NITER = 10

_cache = {}


def _build_toeplitz(psf):
    Wc = np.zeros((P, P), dtype=np.float64)
    j = np.arange(P)[:, None]
    i = np.arange(P)[None, :]
    k = j - i + PAD
    m = (k >= 0) & (k < KTAPS)
    Wc[m] = psf[k[m]]
    WL = np.zeros((32, 32), dtype=np.float64)   # rhs = prev block parts [96,128)
    jj = np.arange(32)[:, None]
    ii = np.arange(32)[None, :]
    k = (96 + jj - 128) - ii + PAD
    m = (k >= 0) & (k < KTAPS)
    WL[m] = psf[k[m]]
    WR = np.zeros((32, 32), dtype=np.float64)   # rhs = next block parts [0,32)
    k = (jj + 128) - (96 + ii) + PAD
    m = (k >= 0) & (k < KTAPS)
    WR[m] = psf[k[m]]
    return Wc, WL, WR


def _wpack(psf):
    """[P, 256] fp16: cols 0:128 = center Toeplitz; cols 128:256 = halo
    weight usable BOTH as one combined [128,128] matmul (against an H-pack
    tile whose rows 0:32 = next-block rows 0:32, rows 96:128 = prev-block
    rows 96:128, middle zero) AND as two separate 32-row matmuls
    (w[0:32,224:256]=WR -> out 96:128; w[96:128,128:160]=WL -> out 0:32)."""
    Wc, WL, WR = _build_toeplitz(psf)
    w = np.zeros((P, 256), dtype=np.float16)
    w[:, 0:128] = Wc
    w[0:32, 128 + 96:128 + 128] = WR
    w[96:128, 128 + 0:128 + 32] = WL
    return w


def _r0pack(psf64):
    """r0[p, t] = 1 / (conv1d(0.5*ones, psf)[128t+p] + EPS)."""
    ones = np.full((1, L), 0.5, dtype=np.float64)
    xp = np.pad(ones, ((0, 0), (PAD, PAD)))
    sc = np.zeros((1, L), dtype=np.float64)
    for k in range(KTAPS):
        sc += xp[:, k:k + L] * psf64[k]
    r = 1.0 / (sc[0] + EPS)
    return r.reshape(NT, P).T.astype(np.float32)


def _build(psf64, alpha64):
    import concourse.bass as bass
    import concourse.tile as tile
    from concourse import mybir
    import bass_rust

    F32 = mybir.dt.float32
    F16 = mybir.dt.float16

    class SafeTC(tile.TileContext):
        # this walrus build rejects >1 sync wait per CTRL-class instruction
        def _drain_and_barrier(self, tick_clock, wait_clock):
            gc = tick_clock.global_clock
            for i in range(len(gc)):
                if gc[i] > 0:
                    di = self.nc.sync.drain()
                    pc = bass_rust.VectorClock()
                    pc.require_at_least(i, gc[i])
                    wait_clock.add_sem_waits(di.ins, bass_rust.ScopedClock({None: pc}))
            self.nc.all_engine_barrier()
            popped = self.nc._tile_sem_poison_stack.pop()
            assert popped is self._sem_poison
            self.nc.clear_and_free_semaphores(list(self.sems.allocated().values()))
            self.nc.all_engine_barrier()

    def split_multi_waits(nc, max_waits=1):
        n_fixed = 0
        uid = [0]
        for f in nc.m.functions:
            for bb in f.blocks:
                out = []
                changed = False
                for inst in bb.instructions:
                    si = inst.sync_info
                    if si is not None:
                        sems = [w for w in si.on_wait
                                if str(getattr(w, "sync_type", "")) == "semaphore"]
                        other = [w for w in si.on_wait if w not in sems]
                        if len(sems) > max_waits:
                            keep = sems[-max_waits:]
                            for w in sems[:-max_waits]:
                                nop = mybir.InstNoOp(
                                    name=f"waitsplit_{uid[0]}", ins=[], outs=[])
                                uid[0] += 1
                                nop.engine = inst.engine
                                nop.sync_info = mybir.SyncInfo(
                                    on_wait=[w], on_update=[])
                                out.append(nop)
                            inst.sync_info = mybir.SyncInfo(
                                on_wait=other + keep,
                                on_update=list(si.on_update))
                            n_fixed += 1
                            changed = True
                    out.append(inst)
                if changed:
                    try:
                        bb.instructions = out
                    except Exception:
                        bb.instructions.clear()
                        bb.instructions.extend(out)
        return n_fixed

    def act_raw(nc, out, in_, func, bias=0.0, scale=1.0):
        eng = nc.scalar
        ins = [eng.lower_ap(in_),
               mybir.ImmediateValue(dtype=F32, value=float(bias)),
               mybir.ImmediateValue(dtype=F32, value=float(scale)),
               mybir.ImmediateValue(dtype=F32, value=0.0)]
        return eng.add_instruction(mybir.InstActivation(
            name=nc.get_next_instruction_name(), func=func, ins=ins,
            outs=[eng.lower_ap(out)]))

    alpha_is_one = bool(np.all(alpha64 == 1.0))

    nc = bass.Bass("TRN2", target_bir_lowering=False, debug=False,
                   num_devices=N_CORES)
    mT_in = nc.dram_tensor("mT", [P, NT, BC], F16, kind="ExternalInput")
    w1_in = nc.dram_tensor("w1", [P, 256], F16, kind="ExternalInput")
    w2_in = nc.dram_tensor("w2", [P, 256], F16, kind="ExternalInput")
    r0_in = nc.dram_tensor("r0", [P, NT], F32, kind="ExternalInput")
    y_out = nc.dram_tensor("y", [P, NT, BC], F16, kind="ExternalOutput")

    Rec = mybir.ActivationFunctionType.Reciprocal
    Ln = mybir.ActivationFunctionType.Ln
    Exp = mybir.ActivationFunctionType.Exp

    RING = 16                 # H1 ring columns
    CH = 4                    # blocks per H1 bulk-copy chunk

    def conv1_block(psum, w, s, h1ring, t):
        """Center matmul + ONE combined-halo matmul against the H1 ring
        column (rows 0:32 = s rows 0:32 of block t+1, rows 96:128 =
        s rows 96:128 of block t-1, middle rows permanently zero)."""
        r = t % RING
        nc.tensor.matmul(psum, w[:, 0:128], s[:, t, :],
                         start=True, stop=False)
        if t == 0:
            nc.tensor.matmul(psum, w[0:32, 128:256], h1ring[0:32, r, :],
                             start=False, stop=True)
        elif t == NT - 1:
            nc.tensor.matmul(psum, w[96:128, 128:256],
                             h1ring[96:128, r, :],
                             start=False, stop=True, tile_position=(96, 0))
        else:
            nc.tensor.matmul(psum, w[:, 128:256], h1ring[:, r, :],
                             start=False, stop=True)

    def conv2_block(psum, w, rat, t):
        """Center + two 32-row halo matmuls reading the ratio tiles'
        partition subranges directly; halos go to disjoint PE quadrants."""
        last = "R" if t < NT - 1 else "L"
        nc.tensor.matmul(psum, w[:, 0:128], rat(t),
                         start=True, stop=False)
        if t > 0:
            nc.tensor.matmul(psum[0:32, :], w[96:128, 128:160],
                             rat(t - 1)[96:128, :], start=False,
                             stop=(last == "L"), tile_position=(96, 0))
        if t < NT - 1:
            nc.tensor.matmul(psum[96:128, :], w[0:32, 224:256],
                             rat(t + 1)[0:32, :], start=False,
                             stop=(last == "R"), tile_position=(0, 96))

    def h1_chunk(h1ring, s, c):
        """Bulk-copy H1 ring slots for blocks [4c, 4c+4): same-partition
        SBUF->SBUF DMAs (one per side), contiguous 4KB per partition."""
        t0, t1 = CH * c, min(CH * (c + 1), NT)
        r0c = (CH * c) % RING
        lo, hi = t0 + 1, min(t1 + 1, NT)
        if hi > lo:
            nc.sync.dma_start(h1ring[0:32, r0c:r0c + (hi - lo), :],
                              s[0:32, lo:hi, :])
        lo, hi = max(t0 - 1, 0), t1 - 1
        if hi > lo:
            rb = r0c if t0 > 0 else r0c + 1
            nc.sync.dma_start(h1ring[96:128, rb:rb + (hi - lo), :],
                              s[96:128, lo:hi, :])

    with SafeTC(nc) as tc:
        with tc.tile_pool(name="wpool", bufs=1) as wpool, \
             tc.tile_pool(name="mpool", bufs=1) as mpool, \
             tc.tile_pool(name="spool", bufs=1) as spool:
            w1 = wpool.tile([P, 256], F16)
            nc.sync.dma_start(w1[:], w1_in[:])
            w2 = wpool.tile([P, 256], F16)
            nc.sync.dma_start(w2[:], w2_in[:])
            r0 = wpool.tile([P, NT], F32)
            nc.sync.dma_start(r0[:], r0_in[:])
            mT = mpool.tile([P, NT, BC], F16)
            # chunked loads: iter0 can start on chunk 0 while 1-3 stream in
            for q in range(4):
                nc.sync.dma_start(mT[:, q * 16:(q + 1) * 16, :],
                                  mT_in[:, q * 16:(q + 1) * 16, :])
            s = spool.tile([P, NT, BC], F16)
            nc.vector.memset(s[:], 0.5)
            h1r = spool.tile([P, RING, BC], F16)
            nc.vector.memset(h1r[:], 0.0)

            # ---- RL iterations (block pairs u=2j, v=2j+1) ----
            with tc.tile_pool(name="ratio", bufs=6) as rpool, \
                 tc.tile_pool(name="rtile", bufs=4) as rtp, \
                 tc.tile_pool(name="psum", bufs=4, space="PSUM") as pp:
                for it in range(NITER):
                    ratio_pairs = [None] * NP

                    def rat(t):
                        return ratio_pairs[t // 2][:, t % 2, :]

                    def _ratio_pair(j):
                        u = 2 * j
                        ra = rpool.tile([P, 2, BC], F16, tag="ra")
                        if it == 0:
                            # s == 0.5 everywhere: conv(s)+EPS is a per-l
                            # constant; r0 = 1/that, precomputed on host.
                            for k in (0, 1):
                                nc.vector.tensor_scalar(
                                    out=ra[:, k, :], in0=mT[:, u + k, :],
                                    scalar1=r0[:, u + k:u + k + 1],
                                    scalar2=None,
                                    op0=mybir.AluOpType.mult)
                        else:
                            ps = pp.tile([P, 2, BC], mybir.dt.float32,
                                         tag="ps")
                            conv1_block(ps[:, 0, :], w1, s, h1r, u)
                            conv1_block(ps[:, 1, :], w1, s, h1r, u + 1)
                            rt = rtp.tile([P, 2, BC], F16, tag="rt")
                            act_raw(nc, rt[:], ps[:], Rec, bias=EPS)
                            nc.vector.tensor_mul(ra[:], mT[:, u:u + 2, :],
                                                 rt[:])
                        ratio_pairs[j] = ra

                    def _conv2_update_pair(j):
                        u = 2 * j
                        ps = pp.tile([P, 2, BC], mybir.dt.float32, tag="ps")
                        conv2_block(ps[:, 0, :], w2, rat, u)
                        conv2_block(ps[:, 1, :], w2, rat, u + 1)
                        if alpha_is_one:
                            if j % 2 == 0:
                                # DVE fused: s = (psum + EPS) * s, PSUM 1x
                                nc.vector.scalar_tensor_tensor(
                                    out=s[:, u:u + 2, :], in0=ps[:],
                                    scalar=EPS, in1=s[:, u:u + 2, :],
                                    op0=mybir.AluOpType.add,
                                    op1=mybir.AluOpType.mult)
                            else:
                                # ACT evacuates PSUM (+EPS), DVE fp16 2x mul
                                cp = rtp.tile([P, 2, BC], F16, tag="cp")
                                act_raw(nc, cp[:], ps[:],
                                        mybir.ActivationFunctionType.Copy,
                                        bias=EPS)
                                nc.vector.tensor_mul(s[:, u:u + 2, :],
                                                     s[:, u:u + 2, :], cp[:])
                        else:
                            lg = rtp.tile([P, 2, BC], F32, tag="lg")
                            act_raw(nc, lg[:], ps[:], Ln, bias=EPS)
                            cp = rtp.tile([P, 2, BC], F16, tag="cp")
                            act_raw(nc, cp[:], lg[:], Exp,
                                    scale=float(alpha64[it]))
                            nc.vector.tensor_mul(s[:, u:u + 2, :],
                                                 s[:, u:u + 2, :], cp[:])
                        # stream the finished s quarter out during iter 9
                        if it == NITER - 1 and (j + 1) % 8 == 0:
                            q = j // 8
                            nc.sync.dma_start(
                                y_out[:, q * 16:(q + 1) * 16, :],
                                s[:, q * 16:(q + 1) * 16, :])

                    # software-pipelined emission (per pair step):
                    # H1 chunks ~6 blocks ahead, ratio 2 pairs ahead.
                    if it > 0:
                        h1_chunk(h1r, s, 0)
                        h1_chunk(h1r, s, 1)
                    _ratio_pair(0)
                    _ratio_pair(1)
                    for j in range(NP):
                        if it > 0 and (j + 4) % 2 == 0:
                            c = (j + 4) // 2
                            if c < NT // CH:
                                h1_chunk(h1r, s, c)
                        if j + 2 < NP:
                            _ratio_pair(j + 2)
                        _conv2_update_pair(j)

    split_multi_waits(nc)
    return nc


def _make_in_maps(m, psf, alpha):
    m = np.asarray(m)
    psf64 = np.asarray(psf, dtype=np.float64)
    w1 = _wpack(psf64)
    w2 = _wpack(psf64[::-1])
    r0 = _r0pack(psf64)
    in_maps = []
    for c in range(N_CORES):
        mc = m[c * BC:(c + 1) * BC].astype(np.float16)      # [BC, L]
        mT = np.ascontiguousarray(
            mc.reshape(BC, NT, P).transpose(2, 1, 0))        # [P, NT, BC]
        in_maps.append({"mT": mT, "w1": w1, "w2": w2, "r0": r0})
    return in_maps


def kernel(m, psf, alpha):
    m = np.asarray(m)
    psf64 = np.asarray(psf, dtype=np.float64)
    alpha64 = np.asarray(alpha, dtype=np.float64)
    key = hashlib.sha256(
        psf64.tobytes() + alpha64.tobytes() + str(m.shape).encode()).hexdigest()
    if key not in _cache:
        _cache[key] = _build(psf64, alpha64)
    nc = _cache[key]

    from concourse.bass_utils import run_bass_kernel_spmd
    in_maps = _make_in_maps(m, psf, alpha)
    res = run_bass_kernel_spmd(nc, in_maps, core_ids=list(range(N_CORES)))
    outs = []
    for c in range(N_CORES):
        yT = res.results[c]["y"]                             # [P, NT, BC] fp16
        outs.append(np.asarray(yT).transpose(2, 1, 0).reshape(BC, L))
    return np.concatenate(outs, axis=0).astype(np.float32)


# revision 15
# speedup vs baseline: 2.3854x; 1.0134x over previous
"""Deep Richardson-Lucy deconvolution on 8 Trainium2 NeuronCores.

Strategy (per core, data-parallel batch shard of 512 rows):
- Everything lives in SBUF in a TRANSPOSED fp16 layout: [l on partitions
  (64 blocks of 128), batch on the free dim (512)]. The transpose/cast of
  m happens on the HOST (same fp16 rounding as on-chip), and the output
  un-transpose/fp32-cast also happens on the host, so the device does
  zero staging work: 4 chunked input DMAs, 4 chunked output DMAs.
- conv1d(K=31, zero-pad) == banded-Toeplitz matmul per 128-l block.
  conv1 (input s): center [128,128] matmul + ONE combined-halo matmul
  against an H1 ring tile filled by bulk same-partition SBUF->SBUF DMAs
  (rows 0:32 = next block's rows 0:32, rows 96:128 = prev block's rows
  96:128, middle rows permanently zero x zero weights).
  conv2 (input ratio): center + two 32-row halo matmuls reading the
  ratio tiles' partition subranges directly; the two halos land in
  disjoint PE quadrants (tile_position) and stream concurrently.
- Elementwise ops run on PAIRS of blocks (2-bank PSUM tiles, free size
  1024) to amortize per-instruction overhead and halve semaphore waits:
  r = ACT.Reciprocal(psum_pair + EPS); ratio = m * r (DVE fp16 2x);
  s *= (psum_pair + EPS) alternating DVE-stt / ACT-copy+DVE-mult.
- Deep software pipelining (ratio 2 pairs ahead, H1 chunks ~6 blocks
  ahead) keeps the PE 100% busy at full stream rate.
"""
import hashlib
import numpy as np

EPS = 1e-6
P = 128
KTAPS = 31
PAD = 15
B_FULL, L = 4096, 8192
N_CORES = 8
BC = B_FULL // N_CORES          # 512 batch rows per core
NT = L // P                     # 64 l-blocks
NP = NT // 2                    # 32 block pairs
NITER = 10

_cache = {}


def _build_toeplitz(psf):
    Wc = np.zeros((P, P), dtype=np.float64)
    j = np.arange(P)[:, None]
    i = np.arange(P)[None, :]
    k = j - i + PAD
    m = (k >= 0) & (k < KTAPS)
    Wc[m] = psf[k[m]]
    WL = np.zeros((32, 32), dtype=np.float64)   # rhs = prev block parts [96,128)
    jj = np.arange(32)[:, None]
    ii = np.arange(32)[None, :]
    k = (96 + jj - 128) - ii + PAD
    m = (k >= 0) & (k < KTAPS)
    WL[m] = psf[k[m]]
    WR = np.zeros((32, 32), dtype=np.float64)   # rhs = next block parts [0,32)
    k = (jj + 128) - (96 + ii) + PAD
    m = (k >= 0) & (k < KTAPS)
    WR[m] = psf[k[m]]
    return Wc, WL, WR


def _wpack(psf):
    """[P, 256] fp16: cols 0:128 = center Toeplitz; cols 128:256 = halo
    weight usable BOTH as one combined [128,128] matmul (against an H-pack
    tile whose rows 0:32 = next-block rows 0:32, rows 96:128 = prev-block
    rows 96:128, middle zero) AND as two separate 32-row matmuls
    (w[0:32,224:256]=WR -> out 96:128; w[96:128,128:160]=WL -> out 0:32)."""
    Wc, WL, WR = _build_toeplitz(psf)
    w = np.zeros((P, 256), dtype=np.float16)
    w[:, 0:128] = Wc
    w[0:32, 128 + 96:128 + 128] = WR
    w[96:128, 128 + 0:128 + 32] = WL
    return w


def _r0pack(psf64):
    """r0[p, t] = 1 / (conv1d(0.5*ones, psf)[128t+p] + EPS)."""
    ones = np.full((1, L), 0.5, dtype=np.float64)
    xp = np.pad(ones, ((0, 0), (PAD, PAD)))
    sc = np.zeros((1, L), dtype=np.float64)
    for k in range(KTAPS):
        sc += xp[:, k:k + L] * psf64[k]
    r = 1.0 / (sc[0] + EPS)
    return r.reshape(NT, P).T.astype(np.float32)


def _build(psf64, alpha64):
    import concourse.bass as bass
    import concourse.tile as tile
    from concourse import mybir
    import bass_rust

    F32 = mybir.dt.float32
    F16 = mybir.dt.float16

    class SafeTC(tile.TileContext):
        # this walrus build rejects >1 sync wait per CTRL-class instruction
        def _drain_and_barrier(self, tick_clock, wait_clock):
            gc = tick_clock.global_clock
            for i in range(len(gc)):
                if gc[i] > 0:
                    di = self.nc.sync.drain()
                    pc = bass_rust.VectorClock()
                    pc.require_at_least(i, gc[i])
                    wait_clock.add_sem_waits(di.ins, bass_rust.ScopedClock({None: pc}))
            self.nc.all_engine_barrier()
            popped = self.nc._tile_sem_poison_stack.pop()
            assert popped is self._sem_poison
            self.nc.clear_and_free_semaphores(list(self.sems.allocated().values()))
            self.nc.all_engine_barrier()

    def split_multi_waits(nc, max_waits=1):
        n_fixed = 0
        uid = [0]
        for f in nc.m.functions:
            for bb in f.blocks:
                out = []
                changed = False
                for inst in bb.instructions:
                    si = inst.sync_info
                    if si is not None:
                        sems = [w for w in si.on_wait
                                if str(getattr(w, "sync_type", "")) == "semaphore"]
                        other = [w for w in si.on_wait if w not in sems]
                        if len(sems) > max_waits:
                            keep = sems[-max_waits:]
                            for w in sems[:-max_waits]:
                                nop = mybir.InstNoOp(
                                    name=f"waitsplit_{uid[0]}", ins=[], outs=[])
                                uid[0] += 1
                                nop.engine = inst.engine
                                nop.sync_info = mybir.SyncInfo(
                                    on_wait=[w], on_update=[])
                                out.append(nop)
                            inst.sync_info = mybir.SyncInfo(
                                on_wait=other + keep,
                                on_update=list(si.on_update))
                            n_fixed += 1
                            changed = True
                    out.append(inst)
                if changed:
                    try:
                        bb.instructions = out
                    except Exception:
                        bb.instructions.clear()
                        bb.instructions.extend(out)
        return n_fixed

    def act_raw(nc, out, in_, func, bias=0.0, scale=1.0):
        eng = nc.scalar
        ins = [eng.lower_ap(in_),
               mybir.ImmediateValue(dtype=F32, value=float(bias)),
               mybir.ImmediateValue(dtype=F32, value=float(scale)),
               mybir.ImmediateValue(dtype=F32, value=0.0)]
        return eng.add_instruction(mybir.InstActivation(
            name=nc.get_next_instruction_name(), func=func, ins=ins,
            outs=[eng.lower_ap(out)]))

    alpha_is_one = bool(np.all(alpha64 == 1.0))

    nc = bass.Bass("TRN2", target_bir_lowering=False, debug=False,
                   num_devices=N_CORES)
    mT_in = nc.dram_tensor("mT", [P, NT, BC], F16, kind="ExternalInput")
    w1_in = nc.dram_tensor("w1", [P, 256], F16, kind="ExternalInput")
    w2_in = nc.dram_tensor("w2", [P, 256], F16, kind="ExternalInput")
    r0_in = nc.dram_tensor("r0", [P, NT], F32, kind="ExternalInput")
    y_out = nc.dram_tensor("y", [P, NT, BC], F16, kind="ExternalOutput")

    Rec = mybir.ActivationFunctionType.Reciprocal
    Ln = mybir.ActivationFunctionType.Ln
    Exp = mybir.ActivationFunctionType.Exp

    RING = 16                 # H1 ring columns
    CH = 4                    # blocks per H1 bulk-copy chunk

    def conv1_block(psum, w, s, h1ring, t):
        """Center matmul + ONE combined-halo matmul against the H1 ring
        column (rows 0:32 = s rows 0:32 of block t+1, rows 96:128 =
        s rows 96:128 of block t-1, middle rows permanently zero)."""
        r = t % RING
        nc.tensor.matmul(psum, w[:, 0:128], s[:, t, :],
                         start=True, stop=False)
        if t == 0:
            nc.tensor.matmul(psum, w[0:32, 128:256], h1ring[0:32, r, :],
                             start=False, stop=True)
        elif t == NT - 1:
            nc.tensor.matmul(psum, w[96:128, 128:256],
                             h1ring[96:128, r, :],
                             start=False, stop=True, tile_position=(96, 0))
        else:
            nc.tensor.matmul(psum, w[:, 128:256], h1ring[:, r, :],
                             start=False, stop=True)

    def conv2_block(psum, w, rat, t):
        """Center + two 32-row halo matmuls reading the ratio tiles'
        partition subranges directly; halos go to disjoint PE quadrants."""
        last = "R" if t < NT - 1 else "L"
        nc.tensor.matmul(psum, w[:, 0:128], rat(t),
                         start=True, stop=False)
        if t > 0:
            nc.tensor.matmul(psum[0:32, :], w[96:128, 128:160],
                             rat(t - 1)[96:128, :], start=False,
                             stop=(last == "L"), tile_position=(96, 0))
        if t < NT - 1:
            nc.tensor.matmul(psum[96:128, :], w[0:32, 224:256],
                             rat(t + 1)[0:32, :], start=False,
                             stop=(last == "R"), tile_position=(0, 96))

    def h1_chunk(h1ring, s, c):
        """Bulk-copy H1 ring slots for blocks [4c, 4c+4): same-partition
        SBUF->SBUF DMAs (one per side), contiguous 4KB per partition."""
        t0, t1 = CH * c, min(CH * (c + 1), NT)
        r0c = (CH * c) % RING
        lo, hi = t0 + 1, min(t1 + 1, NT)
        if hi > lo:
            nc.sync.dma_start(h1ring[0:32, r0c:r0c + (hi - lo), :],
                              s[0:32, lo:hi, :])
        lo, hi = max(t0 - 1, 0), t1 - 1
        if hi > lo:
            rb = r0c if t0 > 0 else r0c + 1
            nc.sync.dma_start(h1ring[96:128, rb:rb + (hi - lo), :],
                              s[96:128, lo:hi, :])

    with SafeTC(nc) as tc:
        with tc.tile_pool(name="wpool", bufs=1) as wpool, \
             tc.tile_pool(name="mpool", bufs=1) as mpool, \
             tc.tile_pool(name="spool", bufs=1) as spool:
            w1 = wpool.tile([P, 256], F16)
            nc.sync.dma_start(w1[:], w1_in[:])
            w2 = wpool.tile([P, 256], F16)
            nc.sync.dma_start(w2[:], w2_in[:])
            r0 = wpool.tile([P, NT], F32)
            nc.sync.dma_start(r0[:], r0_in[:])
            mT = mpool.tile([P, NT, BC], F16)
            # chunked loads: iter0 can start on chunk 0 while 1-3 stream in
            for q in range(4):
                nc.sync.dma_start(mT[:, q * 16:(q + 1) * 16, :],
                                  mT_in[:, q * 16:(q + 1) * 16, :])
            s = spool.tile([P, NT, BC], F16)
            # quarter memsets on the idle Pool engine: startup overlaps the
            # mT chunk loads, and early updates only wait on quarter 0.
            for q in range(4):
                nc.gpsimd.memset(s[:, q * 16:(q + 1) * 16, :], 0.5)
            h1r = spool.tile([P, RING, BC], F16)
            nc.gpsimd.memset(h1r[:], 0.0)

            # ---- RL iterations (block pairs u=2j, v=2j+1) ----
            with tc.tile_pool(name="ratio", bufs=6) as rpool, \
                 tc.tile_pool(name="rtile", bufs=4) as rtp, \
                 tc.tile_pool(name="psum", bufs=4, space="PSUM") as pp:
                for it in range(NITER):
                    ratio_pairs = [None] * NP

                    def rat(t):
                        return ratio_pairs[t // 2][:, t % 2, :]

                    def _ratio_pair(j):
                        u = 2 * j
                        ra = rpool.tile([P, 2, BC], F16, tag="ra")
                        if it == 0:
                            # s == 0.5 everywhere: conv(s)+EPS is a per-l
                            # constant; r0 = 1/that, precomputed on host.
                            for k in (0, 1):
                                nc.vector.tensor_scalar(
                                    out=ra[:, k, :], in0=mT[:, u + k, :],
                                    scalar1=r0[:, u + k:u + k + 1],
                                    scalar2=None,
                                    op0=mybir.AluOpType.mult)
                        else:
                            ps = pp.tile([P, 2, BC], mybir.dt.float32,
                                         tag="ps")
                            conv1_block(ps[:, 0, :], w1, s, h1r, u)
                            conv1_block(ps[:, 1, :], w1, s, h1r, u + 1)
                            rt = rtp.tile([P, 2, BC], F16, tag="rt")
                            act_raw(nc, rt[:], ps[:], Rec, bias=EPS)
                            nc.vector.tensor_mul(ra[:], mT[:, u:u + 2, :],
                                                 rt[:])
                        ratio_pairs[j] = ra

                    def _conv2_update_pair(j):
                        u = 2 * j
                        ps = pp.tile([P, 2, BC], mybir.dt.float32, tag="ps")
                        conv2_block(ps[:, 0, :], w2, rat, u)
                        conv2_block(ps[:, 1, :], w2, rat, u + 1)
                        if alpha_is_one:
                            if j % 2 == 0:
                                # DVE fused: s = (psum + EPS) * s, PSUM 1x
                                nc.vector.scalar_tensor_tensor(
                                    out=s[:, u:u + 2, :], in0=ps[:],
                                    scalar=EPS, in1=s[:, u:u + 2, :],
                                    op0=mybir.AluOpType.add,
                                    op1=mybir.AluOpType.mult)
                            else:
                                # ACT evacuates PSUM (+EPS), DVE fp16 2x mul
                                cp = rtp.tile([P, 2, BC], F16, tag="cp")
                                act_raw(nc, cp[:], ps[:],
                                        mybir.ActivationFunctionType.Copy,
                                        bias=EPS)
                                nc.vector.tensor_mul(s[:, u:u + 2, :],
                                                     s[:, u:u + 2, :], cp[:])
                        else:
                            lg = rtp.tile([P, 2, BC], F32, tag="lg")
                            act_raw(nc, lg[:], ps[:], Ln, bias=EPS)
                            cp = rtp.tile([P, 2, BC], F16, tag="cp")
                            act_raw(nc, cp[:], lg[:], Exp,
                                    scale=float(alpha64[it]))
                            nc.vector.tensor_mul(s[:, u:u + 2, :],
                                                 s[:, u:u + 2, :], cp[:])
                        # stream the finished s quarter out during iter 9
                        if it == NITER - 1 and (j + 1) % 8 == 0:
                            q = j // 8
                            nc.sync.dma_start(
                                y_out[:, q * 16:(q + 1) * 16, :],
                                s[:, q * 16:(q + 1) * 16, :])

                    # software-pipelined emission (per pair step):
                    # H1 chunks ~6 blocks ahead, ratio 2 pairs ahead.
                    if it > 0:
                        h1_chunk(h1r, s, 0)
                        h1_chunk(h1r, s, 1)
                    _ratio_pair(0)
                    _ratio_pair(1)
                    for j in range(NP):
                        if it > 0 and (j + 4) % 2 == 0:
                            c = (j + 4) // 2
                            if c < NT // CH:
                                h1_chunk(h1r, s, c)
                        if j + 2 < NP:
                            _ratio_pair(j + 2)
                        _conv2_update_pair(j)

    split_multi_waits(nc)
    return nc


def _make_in_maps(m, psf, alpha):
    m = np.asarray(m)
    psf64 = np.asarray(psf, dtype=np.float64)
    w1 = _wpack(psf64)
    w2 = _wpack(psf64[::-1])
    r0 = _r0pack(psf64)
    in_maps = []
    for c in range(N_CORES):
        mc = m[c * BC:(c + 1) * BC].astype(np.float16)      # [BC, L]
        mT = np.ascontiguousarray(
            mc.reshape(BC, NT, P).transpose(2, 1, 0))        # [P, NT, BC]
        in_maps.append({"mT": mT, "w1": w1, "w2": w2, "r0": r0})
    return in_maps


def kernel(m, psf, alpha):
    m = np.asarray(m)
    psf64 = np.asarray(psf, dtype=np.float64)
    alpha64 = np.asarray(alpha, dtype=np.float64)
    key = hashlib.sha256(
        psf64.tobytes() + alpha64.tobytes() + str(m.shape).encode()).hexdigest()
    if key not in _cache:
        _cache[key] = _build(psf64, alpha64)
    nc = _cache[key]

    from concourse.bass_utils import run_bass_kernel_spmd
    in_maps = _make_in_maps(m, psf, alpha)
    res = run_bass_kernel_spmd(nc, in_maps, core_ids=list(range(N_CORES)))
    outs = []
    for c in range(N_CORES):
        yT = res.results[c]["y"]                             # [P, NT, BC] fp16
        outs.append(np.asarray(yT).transpose(2, 1, 0).reshape(BC, L))
    return np.concatenate(outs, axis=0).astype(np.float32)


# revision 20
# speedup vs baseline: 2.5816x; 1.0823x over previous
"""Deep Richardson-Lucy deconvolution on 8 Trainium2 NeuronCores.

Strategy (per core, data-parallel batch shard of 512 rows):
- Everything lives in SBUF in a TRANSPOSED fp16 layout: [l on partitions
  (64 blocks of 128), batch on the free dim (512)]. The transpose/cast of
  m happens on the HOST (same fp16 rounding as on-chip), and the output
  un-transpose/fp32-cast also happens on the host, so the device does
  zero staging work: 4 chunked input DMAs, 4 chunked output DMAs.
- conv1d(K=31, zero-pad) == banded-Toeplitz matmul per 128-l block.
  conv1 (input s): center [128,128] matmul + ONE combined-halo matmul
  against an H1 ring tile filled by bulk same-partition SBUF->SBUF DMAs
  (rows 0:32 = next block's rows 0:32, rows 96:128 = prev block's rows
  96:128, middle rows permanently zero x zero weights).
  conv2 (input ratio): center + two 32-row halo matmuls reading the
  ratio tiles' partition subranges directly; the two halos land in
  disjoint PE quadrants (tile_position) and stream concurrently.
- Elementwise ops run on PAIRS of blocks (2-bank PSUM tiles, free size
  1024) to amortize per-instruction overhead and halve semaphore waits:
  r = ACT.Reciprocal(psum_pair + EPS); ratio = m * r (DVE fp16 2x);
  s *= (psum_pair + EPS) alternating DVE-stt / ACT-copy+DVE-mult.
- Deep software pipelining (ratio 2 pairs ahead, H1 chunks ~6 blocks
  ahead) keeps the PE 100% busy at full stream rate.
"""
import hashlib
import numpy as np

EPS = 1e-6
P = 128
KTAPS = 31
PAD = 15
B_FULL, L = 4096, 8192
N_CORES = 8
BC = B_FULL // N_CORES          # 512 batch rows per core
NT = L // P                     # 64 l-blocks
NP = NT // 2                    # 32 block pairs
NITER = 10

_cache = {}


def _build_toeplitz(psf):
    Wc = np.zeros((P, P), dtype=np.float64)
    j = np.arange(P)[:, None]
    i = np.arange(P)[None, :]
    k = j - i + PAD
    m = (k >= 0) & (k < KTAPS)
    Wc[m] = psf[k[m]]
    WL = np.zeros((32, 32), dtype=np.float64)   # rhs = prev block parts [96,128)
    jj = np.arange(32)[:, None]
    ii = np.arange(32)[None, :]
    k = (96 + jj - 128) - ii + PAD
    m = (k >= 0) & (k < KTAPS)
    WL[m] = psf[k[m]]
    WR = np.zeros((32, 32), dtype=np.float64)   # rhs = next block parts [0,32)
    k = (jj + 128) - (96 + ii) + PAD
    m = (k >= 0) & (k < KTAPS)
    WR[m] = psf[k[m]]
    return Wc, WL, WR


def _wpack(psf):
    """[P, 256] fp16: cols 0:128 = center Toeplitz; cols 128:256 = halo
    weight usable BOTH as one combined [128,128] matmul (against an H-pack
    tile whose rows 0:32 = next-block rows 0:32, rows 96:128 = prev-block
    rows 96:128, middle zero) AND as two separate 32-row matmuls
    (w[0:32,224:256]=WR -> out 96:128; w[96:128,128:160]=WL -> out 0:32)."""
    Wc, WL, WR = _build_toeplitz(psf)
    w = np.zeros((P, 256), dtype=np.float16)
    w[:, 0:128] = Wc
    w[0:32, 128 + 96:128 + 128] = WR
    w[96:128, 128 + 0:128 + 32] = WL
    return w


def _r0pack(psf64):
    """r0[p, t] = 1 / (conv1d(0.5*ones, psf)[128t+p] + EPS)."""
    ones = np.full((1, L), 0.5, dtype=np.float64)
    xp = np.pad(ones, ((0, 0), (PAD, PAD)))
    sc = np.zeros((1, L), dtype=np.float64)
    for k in range(KTAPS):
        sc += xp[:, k:k + L] * psf64[k]
    r = 1.0 / (sc[0] + EPS)
    return r.reshape(NT, P).T.astype(np.float32)


def _build(psf64, alpha64):
    import concourse.bass as bass
    import concourse.tile as tile
    from concourse import mybir
    import bass_rust

    F32 = mybir.dt.float32
    F16 = mybir.dt.float16

    class SafeTC(tile.TileContext):
        # this walrus build rejects >1 sync wait per CTRL-class instruction
        def _drain_and_barrier(self, tick_clock, wait_clock):
            gc = tick_clock.global_clock
            for i in range(len(gc)):
                if gc[i] > 0:
                    di = self.nc.sync.drain()
                    pc = bass_rust.VectorClock()
                    pc.require_at_least(i, gc[i])
                    wait_clock.add_sem_waits(di.ins, bass_rust.ScopedClock({None: pc}))
            self.nc.all_engine_barrier()
            popped = self.nc._tile_sem_poison_stack.pop()
            assert popped is self._sem_poison
            self.nc.clear_and_free_semaphores(list(self.sems.allocated().values()))
            self.nc.all_engine_barrier()

    def split_multi_waits(nc, max_waits=1):
        n_fixed = 0
        uid = [0]
        for f in nc.m.functions:
            for bb in f.blocks:
                out = []
                changed = False
                for inst in bb.instructions:
                    si = inst.sync_info
                    if si is not None:
                        sems = [w for w in si.on_wait
                                if str(getattr(w, "sync_type", "")) == "semaphore"]
                        other = [w for w in si.on_wait if w not in sems]
                        if len(sems) > max_waits:
                            keep = sems[-max_waits:]
                            for w in sems[:-max_waits]:
                                nop = mybir.InstNoOp(
                                    name=f"waitsplit_{uid[0]}", ins=[], outs=[])
                                uid[0] += 1
                                nop.engine = inst.engine
                                nop.sync_info = mybir.SyncInfo(
                                    on_wait=[w], on_update=[])
                                out.append(nop)
                            inst.sync_info = mybir.SyncInfo(
                                on_wait=other + keep,
                                on_update=list(si.on_update))
                            n_fixed += 1
                            changed = True
                    out.append(inst)
                if changed:
                    try:
                        bb.instructions = out
                    except Exception:
                        bb.instructions.clear()
                        bb.instructions.extend(out)
        return n_fixed

    def act_raw(nc, out, in_, func, bias=0.0, scale=1.0):
        eng = nc.scalar
        ins = [eng.lower_ap(in_),
               mybir.ImmediateValue(dtype=F32, value=float(bias)),
               mybir.ImmediateValue(dtype=F32, value=float(scale)),
               mybir.ImmediateValue(dtype=F32, value=0.0)]
        return eng.add_instruction(mybir.InstActivation(
            name=nc.get_next_instruction_name(), func=func, ins=ins,
            outs=[eng.lower_ap(out)]))

    alpha_is_one = bool(np.all(alpha64 == 1.0))

    nc = bass.Bass("TRN2", target_bir_lowering=False, debug=False,
                   num_devices=N_CORES)
    mT_in = nc.dram_tensor("mT", [P, NT, BC], F16, kind="ExternalInput")
    w1_in = nc.dram_tensor("w1", [P, 256], F16, kind="ExternalInput")
    w2_in = nc.dram_tensor("w2", [P, 256], F16, kind="ExternalInput")
    r0_in = nc.dram_tensor("r0", [P, NT], F32, kind="ExternalInput")
    y_out = nc.dram_tensor("y", [P, NT, BC], F16, kind="ExternalOutput")

    Rec = mybir.ActivationFunctionType.Reciprocal
    Ln = mybir.ActivationFunctionType.Ln
    Exp = mybir.ActivationFunctionType.Exp

    RING = 16                 # H1 ring columns
    CH = 4                    # blocks per H1 bulk-copy chunk

    def conv1_block(psum, w, s, h1ring, t):
        """Center matmul + ONE combined-halo matmul against the H1 ring
        column (rows 0:32 = s rows 0:32 of block t+1, rows 96:128 =
        s rows 96:128 of block t-1, middle rows permanently zero)."""
        r = t % RING
        nc.tensor.matmul(psum, w[:, 0:128], s[:, t, :],
                         start=True, stop=False)
        if t == 0:
            nc.tensor.matmul(psum, w[0:32, 128:256], h1ring[0:32, r, :],
                             start=False, stop=True)
        elif t == NT - 1:
            nc.tensor.matmul(psum, w[96:128, 128:256],
                             h1ring[96:128, r, :],
                             start=False, stop=True, tile_position=(96, 0))
        else:
            nc.tensor.matmul(psum, w[:, 128:256], h1ring[:, r, :],
                             start=False, stop=True)

    def conv2_block(psum, w, rat, t):
        """Center + two 32-row halo matmuls reading the ratio tiles'
        partition subranges directly; halos go to disjoint PE quadrants."""
        last = "R" if t < NT - 1 else "L"
        nc.tensor.matmul(psum, w[:, 0:128], rat(t),
                         start=True, stop=False)
        if t > 0:
            nc.tensor.matmul(psum[0:32, :], w[96:128, 128:160],
                             rat(t - 1)[96:128, :], start=False,
                             stop=(last == "L"), tile_position=(96, 0))
        if t < NT - 1:
            nc.tensor.matmul(psum[96:128, :], w[0:32, 224:256],
                             rat(t + 1)[0:32, :], start=False,
                             stop=(last == "R"), tile_position=(0, 96))

    def h1_chunk(h1ring, s, c):
        """Bulk-copy H1 ring slots for blocks [4c, 4c+4): same-partition
        SBUF->SBUF DMAs (one per side), contiguous 4KB per partition."""
        t0, t1 = CH * c, min(CH * (c + 1), NT)
        r0c = (CH * c) % RING
        lo, hi = t0 + 1, min(t1 + 1, NT)
        if hi > lo:
            nc.sync.dma_start(h1ring[0:32, r0c:r0c + (hi - lo), :],
                              s[0:32, lo:hi, :])
        lo, hi = max(t0 - 1, 0), t1 - 1
        if hi > lo:
            rb = r0c if t0 > 0 else r0c + 1
            nc.sync.dma_start(h1ring[96:128, rb:rb + (hi - lo), :],
                              s[96:128, lo:hi, :])

    with SafeTC(nc) as tc:
        with tc.tile_pool(name="wpool", bufs=1) as wpool, \
             tc.tile_pool(name="mpool", bufs=1) as mpool, \
             tc.tile_pool(name="spool", bufs=1) as spool:
            w1 = wpool.tile([P, 256], F16)
            nc.sync.dma_start(w1[:], w1_in[:])
            w2 = wpool.tile([P, 256], F16)
            nc.sync.dma_start(w2[:], w2_in[:])
            r0 = wpool.tile([P, NT], F32)
            nc.sync.dma_start(r0[:], r0_in[:])
            mT = mpool.tile([P, NT, BC], F16)
            # chunked loads across 8 DMA queues: iter0 starts on chunk 0
            # quickly and the full tile lands in ~8us instead of ~25us.
            for q in range(8):
                nc.sync.dma_start(mT[:, q * 8:(q + 1) * 8, :],
                                  mT_in[:, q * 8:(q + 1) * 8, :])
            s = spool.tile([P, NT, BC], F16)
            # quarter memsets on the idle Pool engine: startup overlaps the
            # mT chunk loads, and early updates only wait on quarter 0.
            for q in range(4):
                nc.gpsimd.memset(s[:, q * 16:(q + 1) * 16, :], 0.5)
            h1r = spool.tile([P, RING, BC], F16)
            nc.gpsimd.memset(h1r[:], 0.0)

            # ---- RL iterations (block pairs u=2j; ratio in QUAD tiles so
            # conv2's matmuls depend on 3x fewer producer semaphores) ----
            with tc.tile_pool(name="ratio", bufs=5) as rpool, \
                 tc.tile_pool(name="rtile", bufs=4) as rtp, \
                 tc.tile_pool(name="psum", bufs=4, space="PSUM") as pp:
                for it in range(NITER):
                    NQ = NT // 4
                    ratio_quads = [None] * NQ
                    rt_quads = [None] * NQ

                    def rat(t):
                        return ratio_quads[t // 4][:, t % 4, :]

                    def _conv1_recip_pair(j):
                        # conv1 + reciprocal for pair j, written into the
                        # matching half of the quad rt tile.
                        u = 2 * j
                        q4 = j // 2
                        if j % 2 == 0:
                            rtq = rtp.tile([P, 4, BC], F16, tag="rtq")
                            rt_quads[q4] = rtq
                        ps = pp.tile([P, 2, BC], mybir.dt.float32, tag="ps")
                        conv1_block(ps[:, 0, :], w1, s, h1r, u)
                        conv1_block(ps[:, 1, :], w1, s, h1r, u + 1)
                        half = (j % 2) * 2
                        act_raw(nc, rt_quads[q4][:, half:half + 2, :],
                                ps[:], Rec, bias=EPS)

                    def _ratio_quad(q4):
                        u = 4 * q4
                        ra = rpool.tile([P, 4, BC], F16, tag="ra")
                        if it == 0:
                            # s == 0.5 everywhere: conv(s)+EPS is a per-l
                            # constant; r0 = 1/that, precomputed on host.
                            for k in range(4):
                                nc.vector.tensor_scalar(
                                    out=ra[:, k, :], in0=mT[:, u + k, :],
                                    scalar1=r0[:, u + k:u + k + 1],
                                    scalar2=None,
                                    op0=mybir.AluOpType.mult)
                        else:
                            # ONE fp16 2x DVE multiply per 4 blocks
                            nc.vector.tensor_mul(ra[:], mT[:, u:u + 4, :],
                                                 rt_quads[q4][:])
                        ratio_quads[q4] = ra

                    def _conv2_update_pair(j):
                        u = 2 * j
                        ps = pp.tile([P, 2, BC], mybir.dt.float32, tag="ps")
                        conv2_block(ps[:, 0, :], w2, rat, u)
                        conv2_block(ps[:, 1, :], w2, rat, u + 1)
                        if alpha_is_one:
                            if j % 2 == 0:
                                # DVE fused: s = (psum + EPS) * s, PSUM 1x
                                nc.vector.scalar_tensor_tensor(
                                    out=s[:, u:u + 2, :], in0=ps[:],
                                    scalar=EPS, in1=s[:, u:u + 2, :],
                                    op0=mybir.AluOpType.add,
                                    op1=mybir.AluOpType.mult)
                            else:
                                # ACT evacuates PSUM (+EPS), DVE fp16 2x mul
                                cp = rtp.tile([P, 2, BC], F16, tag="cp")
                                act_raw(nc, cp[:], ps[:],
                                        mybir.ActivationFunctionType.Copy,
                                        bias=EPS)
                                nc.vector.tensor_mul(s[:, u:u + 2, :],
                                                     s[:, u:u + 2, :], cp[:])
                        else:
                            lg = rtp.tile([P, 2, BC], F32, tag="lg")
                            act_raw(nc, lg[:], ps[:], Ln, bias=EPS)
                            cp = rtp.tile([P, 2, BC], F16, tag="cp")
                            act_raw(nc, cp[:], lg[:], Exp,
                                    scale=float(alpha64[it]))
                            nc.vector.tensor_mul(s[:, u:u + 2, :],
                                                 s[:, u:u + 2, :], cp[:])
                        # stream the finished s quarter out during iter 9
                        if it == NITER - 1 and (j + 1) % 8 == 0:
                            q = j // 8
                            nc.sync.dma_start(
                                y_out[:, q * 16:(q + 1) * 16, :],
                                s[:, q * 16:(q + 1) * 16, :])

    # software-pipelined emission (per quad step qq):
                    # H1 chunks ~1 quad ahead of conv1, ratio quads 1 quad
                    # ahead of conv2.
                    def _produce_quad(q4):
                        if it > 0:
                            c = q4 + 2              # H1 chunk, 2 quads ahead
                            if c < NT // CH:
                                h1_chunk(h1r, s, c)
                            _conv1_recip_pair(2 * q4)
                            _conv1_recip_pair(2 * q4 + 1)
                        _ratio_quad(q4)

                    if it > 0:
                        h1_chunk(h1r, s, 0)
                        h1_chunk(h1r, s, 1)
                    _produce_quad(0)
                    _produce_quad(1)
                    for qq in range(NQ):
                        if qq + 2 < NQ:
                            _produce_quad(qq + 2)
                        _conv2_update_pair(2 * qq)
                        _conv2_update_pair(2 * qq + 1)

    split_multi_waits(nc)
    return nc


def _make_in_maps(m, psf, alpha):
    m = np.asarray(m)
    psf64 = np.asarray(psf, dtype=np.float64)
    w1 = _wpack(psf64)
    w2 = _wpack(psf64[::-1])
    r0 = _r0pack(psf64)
    in_maps = []
    for c in range(N_CORES):
        mc = m[c * BC:(c + 1) * BC].astype(np.float16)      # [BC, L]
        mT = np.ascontiguousarray(
            mc.reshape(BC, NT, P).transpose(2, 1, 0))        # [P, NT, BC]
        in_maps.append({"mT": mT, "w1": w1, "w2": w2, "r0": r0})
    return in_maps


def kernel(m, psf, alpha):
    m = np.asarray(m)
    psf64 = np.asarray(psf, dtype=np.float64)
    alpha64 = np.asarray(alpha, dtype=np.float64)
    key = hashlib.sha256(
        psf64.tobytes() + alpha64.tobytes() + str(m.shape).encode()).hexdigest()
    if key not in _cache:
        _cache[key] = _build(psf64, alpha64)
    nc = _cache[key]

    from concourse.bass_utils import run_bass_kernel_spmd
    in_maps = _make_in_maps(m, psf, alpha)
    res = run_bass_kernel_spmd(nc, in_maps, core_ids=list(range(N_CORES)))
    outs = []
    for c in range(N_CORES):
        yT = res.results[c]["y"]                             # [P, NT, BC] fp16
        outs.append(np.asarray(yT).transpose(2, 1, 0).reshape(BC, L))
    return np.concatenate(outs, axis=0).astype(np.float32)
